# revision 66
# baseline (speedup 1.0000x reference)
"""Multi-head attention (B=4, S=2048, d_model=1024, h=16) on 8 TRN2 NeuronCores.

Sharding: data-parallel over batch (4) x tensor-parallel over head-groups (2 x 8
heads, column-split Wq/Wk/Wv, row-split Wo). Each core computes a full (2048,
1024) partial of the output projection for its (batch, head-group); the host
sums the two group partials per batch and adds bo.

Device kernel (identical SPMD program on all 8 cores):
  qT/kT = W @ X.T computed directly in head-major layout (TF32 matmuls at full
  PE rate), scoresT = k @ qT per head with 64x128 row-tiled matmul pairs (two
  heads run concurrently on the two halves of the PE array), one 1024-wide exp
  per double-buffered 2-bank PSUM scores block on the scalar engine, AV as
  [v|1].T @ exps so the softmax denominators fall out of the matmul for free,
  normalization via a 128-lane reciprocal on DMA-transposed sums + gpsimd
  partition-broadcast, then the output projection from the already-transposed
  attention output. Projection sub-blocks and output-projection blocks are
  dripped one per attention group to fill the PE under the ACT-bound exp
  stream.
"""
import ml_dtypes
import numpy as np

import concourse.bacc as bacc
import concourse.mybir as mybir
from concourse.tile import TileContext
from concourse.bass_utils import run_bass_kernel_spmd

P = 128
S = 2048          # sequence length
DM = 1024         # d_model
DG = 512          # dims per head-group (8 heads x 64)
NPAIR = 4         # head pairs per group
NQB = 4           # q blocks of 512
NKT = 16          # key tiles of 128
KT = DM // P      # contraction tiles for projections
KT2 = KT // 2     # DoubleRow chunk-pairs (256 dims per matmul)
WSCALE = 32.0     # fp8 weight prescale (wq/wk/wv x32; folded back via
                  # exp scale for q/k and via wo/32 for v)

F32 = mybir.dt.float32
F32R = mybir.dt.float32r
BF16 = mybir.dt.bfloat16
F8 = mybir.dt.float8e4
I16 = mybir.dt.int16
AF = mybir.ActivationFunctionType
ALU = mybir.AluOpType
DR = mybir.MatmulPerfMode.DoubleRow

# Schraudolph fast-exp in bf16: bitcast(int16(A*x + B)) approximates
# exp(x/8) (the 1/8 score scale is folded into A). C=5.58 centers the
# mantissa-linear ripple (~±3%, cancelled common-mode by per-head
# normalization).
EXPA = 0.125 * 128.0 / np.log(2.0)
EXPB = 127.0 * 128.0 - 5.58


def _build(has_bias):
    nc = bacc.Bacc(None, target_bir_lowering=False)
    xqT = nc.dram_tensor("xqT", [DM, S], BF16, kind="ExternalInput")
    xkT = nc.dram_tensor("xkT", [DM, S], BF16, kind="ExternalInput")
    xvT = nc.dram_tensor("xvT", [DM, S], BF16, kind="ExternalInput")
    wqT = nc.dram_tensor("wqT", [DM, DG], BF16, kind="ExternalInput")
    wkT = nc.dram_tensor("wkT", [DM, DG], BF16, kind="ExternalInput")
    wvT = nc.dram_tensor("wvT", [DM, DG], BF16, kind="ExternalInput")
    woT = nc.dram_tensor("woT", [DG, DM], BF16, kind="ExternalInput")
    if has_bias:
        bq = nc.dram_tensor("bq", [1, DG], BF16, kind="ExternalInput")
        bk = nc.dram_tensor("bk", [1, DG], BF16, kind="ExternalInput")
        bv = nc.dram_tensor("bv", [1, DG], BF16, kind="ExternalInput")
    out = nc.dram_tensor("out", [S, DM], F32, kind="ExternalOutput")

    xT = {"q": xqT, "k": xkT, "v": xvT}

    with TileContext(nc) as tc:
        with tc.tile_pool(name="pres", bufs=1) as pres, \
             tc.tile_pool(name="pw", bufs=3) as pw, \
             tc.tile_pool(name="px", bufs=6) as px, \
             tc.tile_pool(name="pxv", bufs=4) as pxv, \
             tc.tile_pool(name="pex", bufs=3) as pex, \
             tc.tile_pool(name="pexf", bufs=2) as pexf, \
             tc.tile_pool(name="psmall", bufs=2) as psmall, \
             tc.tile_pool(name="pout", bufs=3) as pout, \
             tc.tile_pool(name="ps_proj", bufs=2, space="PSUM") as ps_proj, \
             tc.tile_pool(name="ps_sc", bufs=2, space="PSUM") as ps_sc, \
             tc.tile_pool(name="ps_av", bufs=2, space="PSUM") as ps_av:

            # resident tensors
            qT_sb = [pres.tile([P, S], BF16, name=f"qT{p}")
                     for p in range(NPAIR)]
            kT_sb = [pres.tile([P, S], BF16, name=f"kT{p}")
                     for p in range(NPAIR)]
            v_sb = pres.tile([P, NKT, 8, 65], BF16)
            attn_sb = pres.tile([P, NPAIR, S], BF16)

            # weights: wq/wk/wv are dead after pair 0 and wo is only
            # needed from pair 3, so 3 rotating slots cover all four
            w_dram = {"q": wqT, "k": wkT, "v": wvT}
            w_sb = {}

            def ensure_w(key):
                if key in w_sb:
                    return
                if key == "o":
                    t = pw.tile([P, NPAIR, DM], BF16, tag="w", name="wo")
                    nc.sync.dma_start(
                        t[:], woT.rearrange("(kp p) o -> p kp o", p=P))
                else:
                    t = pw.tile([P, KT, DG], BF16, tag="w", name=f"w{key}")
                    nc.sync.dma_start(
                        t[:],
                        w_dram[key].rearrange("(kt p) n -> p kt n", p=P))
                w_sb[key] = t

            nc.vector.memset(v_sb[:, :, :, 64:65], 1.0)

            if has_bias:
                x9 = pres.tile([P, DG], BF16)      # ones row, rest zero
                xv9 = pres.tile([P, P], BF16)
                w9 = {
                    "q": pres.tile([P, DG], BF16, name="w9q"),
                    "k": pres.tile([P, DG], BF16, name="w9k"),
                    "v": pres.tile([P, DG], BF16, name="w9v"),
                }
                for t in (x9, xv9, w9["q"], w9["k"], w9["v"]):
                    nc.vector.memset(t[:], 0.0)
                nc.vector.memset(x9[0:1, :], 1.0)
                nc.vector.memset(xv9[0:1, :], 1.0)
                for key, d in (("q", bq), ("k", bk), ("v", bv)):
                    nc.sync.dma_start(w9[key][0:1, :], d[:])

            emitted = set()
            queued = set()
            pending = []          # deferred emitters, dripped between groups
            x_tiles = {}

            def load_x(proj, nb, p):
                # per-pair x loads: more DMA traffic (72MB/core total)
                # but it spreads evenly across the whole kernel instead of
                # overloading the first pair's window
                key = ("x", proj, nb, p)
                if key in x_tiles:
                    return x_tiles[key]
                xs = []
                half = (KT + 1) // 2
                for j in range(2):
                    lo = j * half
                    hi = min(KT, lo + half)
                    xt = px.tile([P, half, DG], BF16, tag="x",
                                 name=f"x_{proj}{nb}_{p}_{j}")
                    nc.sync.dma_start(
                        xt[:, 0:hi - lo, :],
                        xT[proj].rearrange("(kt p) s -> p kt s", p=P)
                        [:, lo:hi, nb * DG:(nb + 1) * DG],
                    )
                    xs.append(xt)
                x_tiles[key] = xs
                return xs

            def qk_first(proj, nb, p):
                """First half (kt 0..3) of a q/k projection chain."""
                ensure_w(proj)
                xs = load_x(proj, nb, p)
                ps = ps_proj.tile([P, DG], F32, tag="pp",
                                  name=f"ps_{proj}{nb}_{p}")
                for kt in range(KT // 2):
                    nc.tensor.matmul(
                        ps[:], w_sb[proj][:, kt, p * P:(p + 1) * P],
                        xs[0][:, kt, :],
                        start=(kt == 0), stop=False,
                    )
                return ps, xs

            def qk_second(proj, nb, p, ps, xs):
                """Second half (kt 4..7) + evacuation."""
                dst = qT_sb if proj == "q" else kT_sb
                half = KT // 2
                for kt in range(half, KT):
                    nc.tensor.matmul(
                        ps[:], w_sb[proj][:, kt, p * P:(p + 1) * P],
                        xs[1][:, kt - half, :],
                        start=False,
                        stop=(kt == KT - 1 and not has_bias),
                    )
                if has_bias:
                    nc.tensor.matmul(
                        ps[:], w9[proj][:, p * P:(p + 1) * P], x9[:],
                        start=False, stop=True,
                    )
                nc.vector.tensor_copy(dst[p][:, nb * DG:(nb + 1) * DG], ps[:])

            def qk_subblock(proj, nb, p):
                """Project q or k for seq block nb, one pair."""
                ps, xs = qk_first(proj, nb, p)
                qk_second(proj, nb, p, ps, xs)

            xv_tiles = {}

            def load_xv(m):
                if m in xv_tiles or m >= NKT:
                    return
                xt = pxv.tile([P, KT, P], BF16, tag="xv", name=f"xv{m}")
                nc.sync.dma_start(
                    xt[:],
                    xvT.rearrange("(kt p) s -> p kt s", p=P)
                    [:, :, m * P:(m + 1) * P],
                )
                xv_tiles[m] = xt

            def v_block(m):
                """Project v for key tile m (128 positions, all 8 heads)."""
                ensure_w("v")
                load_xv(m)
                xt = xv_tiles.pop(m)
                load_xv(m + 1)       # prefetch: next chain never heads the
                load_xv(m + 2)       # PE queue waiting on its xv DMA
                ps = ps_proj.tile([P, DG], F32, tag="pp", name=f"ps_v{m}")
                for kt in range(KT):
                    nc.tensor.matmul(
                        ps[:], xt[:, kt, :], w_sb["v"][:, kt, :],
                        start=(kt == 0),
                        stop=(kt == KT - 1 and not has_bias),
                    )
                if has_bias:
                    nc.tensor.matmul(ps[:], xv9[:], w9["v"][:],
                                     start=False, stop=True)
                nc.vector.tensor_copy(
                    v_sb[:, m, :, 0:64],
                    ps[:].rearrange("p (h d) -> p h d", d=64),
                )

            def oproj_subblock(m, n):
                ensure_w("o")
                ps = ps_proj.tile([P, DG], F32, tag="pp", name=f"ps_o{m}_{n}")
                for kp in range(NPAIR):
                    nc.tensor.matmul(
                        ps[:], attn_sb[:, kp, m * P:(m + 1) * P],
                        w_sb["o"][:, kp, n * DG:(n + 1) * DG],
                        start=(kp == 0), stop=(kp == NPAIR - 1),
                    )
                ot = pout.tile([P, DG], F32, tag="ot", name=f"ot{m}_{n}")
                nc.vector.tensor_copy(ot[:], ps[:])
                nc.sync.dma_start(
                    out[m * P:(m + 1) * P, n * DG:(n + 1) * DG], ot[:])

            pending_o = []        # oproj items, dripped after AV

            def queue(key):
                if key not in emitted and key not in queued:
                    queued.add(key)
                    pending.append(key)

            def ensure(key):
                if key in emitted:
                    return
                # a half-open chain shares ps_proj bufs; emitting anything
                # else from the pool in between would recycle its bank
                flush_half()
                emitted.add(key)
                kind = key[0]
                if kind == "v":
                    v_block(key[1])
                else:
                    qk_subblock(*key)

            half_open = []        # qk chain with only kt 0..3 emitted

            def flush_half():
                while half_open:
                    key, ps, xs = half_open.pop(0)
                    qk_second(*key, ps, xs)

            def drip_qk(full=False):
                # chains drip in 4-matmul halves (~850ns) so a drip never
                # overruns the exp window and delays the AV matmuls behind
                # it; full chains where other ps_proj users interleave
                if half_open:
                    key, ps, xs = half_open.pop(0)
                    qk_second(*key, ps, xs)
                    return
                while pending:
                    key = pending.pop(0)
                    if key in emitted:
                        continue
                    # prefetch x for the next queued qk subblock so its MM
                    # chain never heads the PE queue waiting on DMA
                    for nxt in pending:
                        if nxt not in emitted:
                            load_x(nxt[0], nxt[1], nxt[2])
                            break
                    emitted.add(key)
                    if full:
                        qk_subblock(*key)
                    else:
                        ps, xs = qk_first(*key)
                        half_open.append((key, ps, xs))
                    return

            def drip_o():
                if pending_o:
                    m, n = pending_o.pop(0)
                    oproj_subblock(m, n)

            def oproj_chunk(qb):
                """Queue output projection for seq rows qb*512..+512."""
                for mi in range(4):
                    for n in range(2):
                        pending_o.append((4 * qb + mi, n))

            def scores_mm(p, qb, g):
                sc = ps_sc.tile([P, 2, DG], F32, tag="sc",
                                name=f"sc{p}_{qb}_{g}")
                for h in range(2):
                    nc.tensor.matmul(
                        sc[:, h, :],
                        kT_sb[p][64 * h:64 * h + 64, g * P:(g + 1) * P],
                        qT_sb[p][64 * h:64 * h + 64, qb * DG:(qb + 1) * DG],
                        start=True, stop=True,
                        tile_position=(64 * h, 0),
                    )
                return sc

            # attention: pair-outer, q-block, one key tile per group.
            # scores(g+1) are emitted before AV(g) so the PE computes them
            # under exp(g) and the exp cadence stays at the ACT floor.
            for p in range(NPAIR):
                for qb in range(NQB):
                    flush_half()
                    ensure(("q", qb, p))
                    if qb + 1 < NQB:
                        queue(("q", qb + 1, p))
                    elif p + 1 < NPAIR:
                        queue(("q", 0, p + 1))
                    if qb == NQB - 1 and p + 1 < NPAIR:
                        for nb in range(4):
                            queue(("k", nb, p + 1))
                    av = [
                        ps_av.tile([65, DG], F32, tag="av",
                                   name=f"av{p}_{qb}_{h}")
                        for h in range(2)
                    ]
                    ensure(("k", 0, p))
                    ensure(("v", 0))
                    sc_cur = scores_mm(p, qb, 0)
                    for g in range(NKT):
                        if g + 1 < NKT:
                            ensure(("k", (g + 1) // 4, p))
                            ensure(("v", g + 1))
                        # in the ACT-paced middle pairs, the DVE takes head
                        # 1 on odd g via the int16 fast-exp (bitcast to
                        # bf16); p=0/p=3 windows are PE-bound so ACT does
                        # both heads there at full precision
                        fast = p in (1, 2) and g % 2 == 1
                        ex = pex.tile([P, 2, DG], BF16, tag="ex",
                                      name=f"ex{p}_{qb}_{g}")
                        if fast:
                            nc.scalar.activation(ex[:, 0, :],
                                                 sc_cur[:, 0, :], AF.Exp,
                                                 scale=0.125)
                            exf = pexf.tile([P, DG], I16, tag="exf",
                                            name=f"exf{p}_{qb}_{g}")
                            with nc.allow_low_precision(
                                    reason="fast-exp; per-head softmax "
                                    "normalization cancels the bias"):
                                nc.vector.tensor_scalar(
                                    exf[:], sc_cur[:, 1, :], EXPA, EXPB,
                                    ALU.mult, ALU.add)
                            rhs = [ex[:, 0, :], exf[:].bitcast(BF16)]
                        else:
                            nc.scalar.activation(ex[:], sc_cur[:], AF.Exp,
                                                 scale=0.125)
                            rhs = [ex[:, 0, :], ex[:, 1, :]]
                        if g + 1 < NKT:
                            sc_cur = scores_mm(p, qb, g + 1)
                        # one qk drip BEFORE the AV matmuls: AV(g) waits on
                        # exp(g) (ACT, ~1.1us) at the head of the in-order
                        # PE queue, so this fills the stall with projection
                        # work whose inputs are prefetched; oproj drips stay
                        # after AV (their LDW waits on fresh normalization).
                        # p==0 drips full chains: v_blocks interleave there
                        # and would recycle a half-open chain's psum bank
                        drip_qk(full=(p == 0))
                        for h in range(2):
                            nc.tensor.matmul(
                                av[h][:],
                                v_sb[:, g, 2 * p + h, :],
                                rhs[h],
                                start=(g == 0),
                                stop=(g == NKT - 1),
                            )
                        # g >= 3 so an oproj LDW never heads the PE queue
                        # waiting on the previous block's fresh normalization
                        if p == NPAIR - 1 and g >= 3:
                            drip_o()
                            drip_o()
                    # boundary: evacuate unnormalized attn, then normalize:
                    # 1-lane approx reciprocal on the sums row (the matmul
                    # ones-column), partition-broadcast, multiply in place
                    for h in range(2):
                        nc.vector.tensor_copy(
                            attn_sb[64 * h:64 * h + 64, p,
                                    qb * DG:(qb + 1) * DG],
                            av[h][0:64, :],
                        )
                        s0 = psmall.tile([1, DG], F32, tag="s0",
                                         name=f"s0_{qb}_{p}_{h}")
                        nc.vector.tensor_scalar_mul(
                            s0[0:1, :], av[h][64:65, :], 1.0)
                        r1 = psmall.tile([1, DG], F32, tag="r1",
                                         name=f"r1_{qb}_{p}_{h}")
                        with nc.allow_low_precision(
                                reason="softmax denominators, ~51 ULP"):
                            nc.vector.reciprocal_approx_fast(
                                r1[0:1, :], s0[0:1, :])
                        rbc = psmall.tile([P, DG], F32, tag="rbc",
                                          name=f"rbc{qb}_{p}_{h}")
                        nc.gpsimd.partition_broadcast(rbc[:], r1[0:1, :])
                        sl = attn_sb[64 * h:64 * h + 64, p,
                                     qb * DG:(qb + 1) * DG]
                        nc.vector.tensor_tensor(
                            sl, sl, rbc[64 * h:64 * h + 64, :],
                            mybir.AluOpType.mult)
                    if p == NPAIR - 1:
                        oproj_chunk(qb)
            while pending or half_open:
                drip_qk()
            while pending_o:
                drip_o()
    nc.compile()
    return nc


_CACHE = {}


def _get_nc(has_bias):
    if has_bias not in _CACHE:
        _CACHE[has_bias] = _build(has_bias)
    return _CACHE[has_bias]


def _tr(a):
    return np.ascontiguousarray(
        np.asarray(a, dtype=np.float32).T).astype(ml_dtypes.bfloat16)


def _tr8(a, mult=1.0):
    t = np.ascontiguousarray(np.asarray(a, dtype=np.float32).T) * mult
    return np.clip(t, -440.0, 440.0).astype(ml_dtypes.float8_e4m3)


def _run(Q, K, V, Wq, bq, Wk, bk, Wv, bv, Wo, bo, trace=False):
    Q, K, V = (np.asarray(t, np.float32) for t in (Q, K, V))
    Wq, Wk, Wv, Wo = (np.asarray(t, np.float32) for t in (Wq, Wk, Wv, Wo))
    bq, bk, bv, bo = (np.asarray(t, np.float32) for t in (bq, bk, bv, bo))
    B = Q.shape[0]
    has_bias = bool(np.any(bq) or np.any(bk) or np.any(bv))
    nc = _get_nc(has_bias)

    xts = [(_tr(Q[b]), _tr(K[b]), _tr(V[b])) for b in range(B)]
    wts = []
    for g in range(2):
        sl = slice(DG * g, DG * (g + 1))
        wts.append({
            "wqT": _tr(Wq[sl]), "wkT": _tr(Wk[sl]), "wvT": _tr(Wv[sl]),
            "woT": _tr(Wo[:, sl]),
            "bq": np.ascontiguousarray(bq[None, sl]).astype(ml_dtypes.bfloat16),
            "bk": np.ascontiguousarray(bk[None, sl]).astype(ml_dtypes.bfloat16),
            "bv": np.ascontiguousarray(bv[None, sl]).astype(ml_dtypes.bfloat16),
        })
    in_maps = []
    for c in range(8):
        b, g = c // 2, c % 2
        m = {
            "xqT": xts[b][0], "xkT": xts[b][1], "xvT": xts[b][2],
            "wqT": wts[g]["wqT"], "wkT": wts[g]["wkT"],
            "wvT": wts[g]["wvT"], "woT": wts[g]["woT"],
        }
        if has_bias:
            m["bq"] = wts[g]["bq"]
            m["bk"] = wts[g]["bk"]
            m["bv"] = wts[g]["bv"]
        in_maps.append(m)

    res = run_bass_kernel_spmd(nc, in_maps, core_ids=list(range(8)),
                               trace=trace)
    outp = np.empty((B, S, DM), np.float32)
    for b in range(B):
        outp[b] = res.results[2 * b]["out"] + res.results[2 * b + 1]["out"]
    outp += bo[None, None, :]
    return outp, res


def kernel(Q, K, V, Wq, bq, Wk, bk, Wv, bv, Wo, bo):
    outp, _ = _run(Q, K, V, Wq, bq, Wk, bk, Wv, bv, Wo, bo, trace=False)
    return outp



# revision 71
# speedup vs baseline: 1.0997x; 1.0997x over previous
"""Multi-head attention (B=4, S=2048, d_model=1024, h=16) on 8 TRN2 NeuronCores.

Sharding: data-parallel over batch (4) x tensor-parallel over head-groups (2 x 8
heads, column-split Wq/Wk/Wv, row-split Wo). Each core computes a full (2048,
1024) partial of the output projection for its (batch, head-group); the host
sums the two group partials per batch and adds bo.

Device kernel (identical SPMD program on all 8 cores):
  qT/kT = W @ X.T computed directly in head-major layout (TF32 matmuls at full
  PE rate), scoresT = k @ qT per head with 64x128 row-tiled matmul pairs (two
  heads run concurrently on the two halves of the PE array), one 1024-wide exp
  per double-buffered 2-bank PSUM scores block on the scalar engine, AV as
  [v|1].T @ exps so the softmax denominators fall out of the matmul for free,
  normalization via a 128-lane reciprocal on DMA-transposed sums + gpsimd
  partition-broadcast, then the output projection from the already-transposed
  attention output. Projection sub-blocks and output-projection blocks are
  dripped one per attention group to fill the PE under the ACT-bound exp
  stream.
"""
import ml_dtypes
import numpy as np

import concourse.bacc as bacc
import concourse.mybir as mybir
from concourse.tile import TileContext
from concourse.bass_utils import run_bass_kernel_spmd

P = 128
S = 2048          # sequence length
DM = 1024         # d_model
DG = 512          # dims per head-group (8 heads x 64)
NPAIR = 4         # head pairs per group
NQB = 4           # q blocks of 512
NKT = 16          # key tiles of 128
KT = DM // P      # contraction tiles for projections
KT2 = KT // 2     # DoubleRow chunk-pairs (256 dims per matmul)
WSCALE = 32.0     # fp8 weight prescale (wq/wk/wv x32; folded back via
                  # exp scale for q/k and via wo/32 for v)

F32 = mybir.dt.float32
F32R = mybir.dt.float32r
BF16 = mybir.dt.bfloat16
F8 = mybir.dt.float8e4
I16 = mybir.dt.int16
AF = mybir.ActivationFunctionType
ALU = mybir.AluOpType
DR = mybir.MatmulPerfMode.DoubleRow

# Schraudolph fast-exp in bf16: bitcast(int16(A*x + B)) approximates
# exp(x/8) (the 1/8 score scale is folded into A). C=5.58 centers the
# mantissa-linear ripple (~±3%, cancelled common-mode by per-head
# normalization).
EXPA = 0.125 * 128.0 / np.log(2.0)
EXPB = 127.0 * 128.0 - 5.58


def _build(has_bias):
    nc = bacc.Bacc(None, target_bir_lowering=False)
    xqT = nc.dram_tensor("xqT", [DM, S], BF16, kind="ExternalInput")
    xkT = nc.dram_tensor("xkT", [DM, S], BF16, kind="ExternalInput")
    xvT = nc.dram_tensor("xvT", [DM, S], BF16, kind="ExternalInput")
    wqT = nc.dram_tensor("wqT", [DM, DG], BF16, kind="ExternalInput")
    wkT = nc.dram_tensor("wkT", [DM, DG], BF16, kind="ExternalInput")
    wvT = nc.dram_tensor("wvT", [DM, DG], BF16, kind="ExternalInput")
    woT = nc.dram_tensor("woT", [DG, DM], BF16, kind="ExternalInput")
    if has_bias:
        bq = nc.dram_tensor("bq", [1, DG], BF16, kind="ExternalInput")
        bk = nc.dram_tensor("bk", [1, DG], BF16, kind="ExternalInput")
        bv = nc.dram_tensor("bv", [1, DG], BF16, kind="ExternalInput")
    out = nc.dram_tensor("out", [S, DM], BF16, kind="ExternalOutput")

    xT = {"q": xqT, "k": xkT, "v": xvT}

    with TileContext(nc) as tc:
        with tc.tile_pool(name="pres", bufs=1) as pres, \
             tc.tile_pool(name="pw", bufs=3) as pw, \
             tc.tile_pool(name="px", bufs=6) as px, \
             tc.tile_pool(name="pxv", bufs=4) as pxv, \
             tc.tile_pool(name="pex", bufs=3) as pex, \
             tc.tile_pool(name="psmall", bufs=2) as psmall, \
             tc.tile_pool(name="pout", bufs=3) as pout, \
             tc.tile_pool(name="ps_proj", bufs=2, space="PSUM") as ps_proj, \
             tc.tile_pool(name="ps_sc", bufs=2, space="PSUM") as ps_sc, \
             tc.tile_pool(name="ps_av", bufs=2, space="PSUM") as ps_av:

            # resident tensors
            qT_sb = [pres.tile([P, S], BF16, name=f"qT{p}")
                     for p in range(NPAIR)]
            kT_sb = [pres.tile([P, S], BF16, name=f"kT{p}")
                     for p in range(NPAIR)]
            v_sb = pres.tile([P, NKT, 8, 65], BF16)
            attn_sb = pres.tile([P, NPAIR, S], BF16)

            # weights: wq/wk/wv are dead after pair 0 and wo is only
            # needed from pair 3, so 3 rotating slots cover all four
            w_dram = {"q": wqT, "k": wkT, "v": wvT}
            w_sb = {}

            def ensure_w(key):
                if key in w_sb:
                    return
                if key == "o":
                    t = pw.tile([P, NPAIR, DM], BF16, tag="w", name="wo")
                    nc.sync.dma_start(
                        t[:], woT.rearrange("(kp p) o -> p kp o", p=P))
                else:
                    t = pw.tile([P, KT, DG], BF16, tag="w", name=f"w{key}")
                    nc.sync.dma_start(
                        t[:],
                        w_dram[key].rearrange("(kt p) n -> p kt n", p=P))
                w_sb[key] = t

            nc.vector.memset(v_sb[:, :, :, 64:65], 1.0)

            if has_bias:
                x9 = pres.tile([P, DG], BF16)      # ones row, rest zero
                xv9 = pres.tile([P, P], BF16)
                w9 = {
                    "q": pres.tile([P, DG], BF16, name="w9q"),
                    "k": pres.tile([P, DG], BF16, name="w9k"),
                    "v": pres.tile([P, DG], BF16, name="w9v"),
                }
                for t in (x9, xv9, w9["q"], w9["k"], w9["v"]):
                    nc.vector.memset(t[:], 0.0)
                nc.vector.memset(x9[0:1, :], 1.0)
                nc.vector.memset(xv9[0:1, :], 1.0)
                for key, d in (("q", bq), ("k", bk), ("v", bv)):
                    nc.sync.dma_start(w9[key][0:1, :], d[:])

            emitted = set()
            queued = set()
            pending = []          # deferred emitters, dripped between groups
            x_tiles = {}

            def load_x(proj, nb, p):
                # per-pair x loads: more DMA traffic (72MB/core total)
                # but it spreads evenly across the whole kernel instead of
                # overloading the first pair's window
                key = ("x", proj, nb, p)
                if key in x_tiles:
                    return x_tiles[key]
                xs = []
                half = (KT + 1) // 2
                for j in range(2):
                    lo = j * half
                    hi = min(KT, lo + half)
                    xt = px.tile([P, half, DG], BF16, tag="x",
                                 name=f"x_{proj}{nb}_{p}_{j}")
                    nc.sync.dma_start(
                        xt[:, 0:hi - lo, :],
                        xT[proj].rearrange("(kt p) s -> p kt s", p=P)
                        [:, lo:hi, nb * DG:(nb + 1) * DG],
                    )
                    xs.append(xt)
                x_tiles[key] = xs
                return xs

            def qk_first(proj, nb, p):
                """First half (kt 0..3) of a q/k projection chain."""
                ensure_w(proj)
                xs = load_x(proj, nb, p)
                ps = ps_proj.tile([P, DG], F32, tag="pp",
                                  name=f"ps_{proj}{nb}_{p}")
                for kt in range(KT // 2):
                    nc.tensor.matmul(
                        ps[:], w_sb[proj][:, kt, p * P:(p + 1) * P],
                        xs[0][:, kt, :],
                        start=(kt == 0), stop=False,
                    )
                return ps, xs

            def qk_second(proj, nb, p, ps, xs):
                """Second half (kt 4..7) + evacuation."""
                dst = qT_sb if proj == "q" else kT_sb
                half = KT // 2
                for kt in range(half, KT):
                    nc.tensor.matmul(
                        ps[:], w_sb[proj][:, kt, p * P:(p + 1) * P],
                        xs[1][:, kt - half, :],
                        start=False,
                        stop=(kt == KT - 1 and not has_bias),
                    )
                if has_bias:
                    nc.tensor.matmul(
                        ps[:], w9[proj][:, p * P:(p + 1) * P], x9[:],
                        start=False, stop=True,
                    )
                nc.vector.tensor_copy(dst[p][:, nb * DG:(nb + 1) * DG], ps[:])

            def qk_subblock(proj, nb, p):
                """Project q or k for seq block nb, one pair."""
                ps, xs = qk_first(proj, nb, p)
                qk_second(proj, nb, p, ps, xs)

            xv_tiles = {}

            def load_xv(m):
                if m in xv_tiles or m >= NKT:
                    return
                xt = pxv.tile([P, KT, P], BF16, tag="xv", name=f"xv{m}")
                nc.sync.dma_start(
                    xt[:],
                    xvT.rearrange("(kt p) s -> p kt s", p=P)
                    [:, :, m * P:(m + 1) * P],
                )
                xv_tiles[m] = xt

            def v_block(m):
                """Project v for key tile m (128 positions, all 8 heads)."""
                ensure_w("v")
                load_xv(m)
                xt = xv_tiles.pop(m)
                load_xv(m + 1)       # prefetch: next chain never heads the
                load_xv(m + 2)       # PE queue waiting on its xv DMA
                ps = ps_proj.tile([P, DG], F32, tag="pp", name=f"ps_v{m}")
                for kt in range(KT):
                    nc.tensor.matmul(
                        ps[:], xt[:, kt, :], w_sb["v"][:, kt, :],
                        start=(kt == 0),
                        stop=(kt == KT - 1 and not has_bias),
                    )
                if has_bias:
                    nc.tensor.matmul(ps[:], xv9[:], w9["v"][:],
                                     start=False, stop=True)
                nc.vector.tensor_copy(
                    v_sb[:, m, :, 0:64],
                    ps[:].rearrange("p (h d) -> p h d", d=64),
                )

            def oproj_subblock(m, n):
                ensure_w("o")
                ps = ps_proj.tile([P, DG], F32, tag="pp", name=f"ps_o{m}_{n}")
                for kp in range(NPAIR):
                    nc.tensor.matmul(
                        ps[:], attn_sb[:, kp, m * P:(m + 1) * P],
                        w_sb["o"][:, kp, n * DG:(n + 1) * DG],
                        start=(kp == 0), stop=(kp == NPAIR - 1),
                    )
                ot = pout.tile([P, DG], BF16, tag="ot", name=f"ot{m}_{n}")
                nc.vector.tensor_copy(ot[:], ps[:])
                nc.sync.dma_start(
                    out[m * P:(m + 1) * P, n * DG:(n + 1) * DG], ot[:])

            pending_o = []        # oproj items, dripped after AV

            def queue(key):
                if key not in emitted and key not in queued:
                    queued.add(key)
                    pending.append(key)

            def ensure(key):
                if key in emitted:
                    return
                # a half-open chain shares ps_proj bufs; emitting anything
                # else from the pool in between would recycle its bank
                flush_half()
                emitted.add(key)
                kind = key[0]
                if kind == "v":
                    v_block(key[1])
                else:
                    qk_subblock(*key)

            half_open = []        # qk chain with only kt 0..3 emitted

            def flush_half():
                while half_open:
                    key, ps, xs = half_open.pop(0)
                    qk_second(*key, ps, xs)

            def drip_qk(full=False):
                # chains drip in 4-matmul halves (~850ns) so a drip never
                # overruns the exp window and delays the AV matmuls behind
                # it; full chains where other ps_proj users interleave
                if half_open:
                    key, ps, xs = half_open.pop(0)
                    qk_second(*key, ps, xs)
                    return
                while pending:
                    key = pending.pop(0)
                    if key in emitted:
                        continue
                    # prefetch x for the next queued qk subblock so its MM
                    # chain never heads the PE queue waiting on DMA
                    for nxt in pending:
                        if nxt not in emitted:
                            load_x(nxt[0], nxt[1], nxt[2])
                            break
                    emitted.add(key)
                    if full:
                        qk_subblock(*key)
                    else:
                        ps, xs = qk_first(*key)
                        half_open.append((key, ps, xs))
                    return

            def drip_o():
                if pending_o:
                    m, n = pending_o.pop(0)
                    oproj_subblock(m, n)

            def oproj_chunk(qb):
                """Queue output projection for seq rows qb*512..+512."""
                for mi in range(4):
                    for n in range(2):
                        pending_o.append((4 * qb + mi, n))

            def scores_mm(p, qb, g):
                sc = ps_sc.tile([P, 2, DG], F32, tag="sc",
                                name=f"sc{p}_{qb}_{g}")
                for h in range(2):
                    nc.tensor.matmul(
                        sc[:, h, :],
                        kT_sb[p][64 * h:64 * h + 64, g * P:(g + 1) * P],
                        qT_sb[p][64 * h:64 * h + 64, qb * DG:(qb + 1) * DG],
                        start=True, stop=True,
                        tile_position=(64 * h, 0),
                    )
                return sc

            # attention: pair-outer, q-block, one key tile per group.
            # scores(g+1) are emitted before AV(g) so the PE computes them
            # under exp(g) and the exp cadence stays at the ACT floor.
            for p in range(NPAIR):
                for qb in range(NQB):
                    flush_half()
                    ensure(("q", qb, p))
                    if qb + 1 < NQB:
                        queue(("q", qb + 1, p))
                    elif p + 1 < NPAIR:
                        queue(("q", 0, p + 1))
                    if qb == NQB - 1 and p + 1 < NPAIR:
                        for nb in range(4):
                            queue(("k", nb, p + 1))
                    av = [
                        ps_av.tile([65, DG], F32, tag="av",
                                   name=f"av{p}_{qb}_{h}")
                        for h in range(2)
                    ]
                    ensure(("k", 0, p))
                    ensure(("v", 0))
                    sc_cur = scores_mm(p, qb, 0)
                    for g in range(NKT):
                        if g + 1 < NKT:
                            ensure(("k", (g + 1) // 4, p))
                            ensure(("v", g + 1))
                        ex = pex.tile([P, 2, DG], BF16, tag="ex",
                                      name=f"ex{p}_{qb}_{g}")
                        nc.scalar.activation(ex[:], sc_cur[:], AF.Exp,
                                             scale=0.125)
                        rhs = [ex[:, 0, :], ex[:, 1, :]]
                        if g + 1 < NKT:
                            sc_cur = scores_mm(p, qb, g + 1)
                        # one qk drip BEFORE the AV matmuls: AV(g) waits on
                        # exp(g) (ACT, ~1.1us) at the head of the in-order
                        # PE queue, so this fills the stall with projection
                        # work whose inputs are prefetched; oproj drips stay
                        # after AV (their LDW waits on fresh normalization).
                        # p==0 drips full chains: v_blocks interleave there
                        # and would recycle a half-open chain's psum bank
                        drip_qk(full=(p == 0))
                        for h in range(2):
                            nc.tensor.matmul(
                                av[h][:],
                                v_sb[:, g, 2 * p + h, :],
                                rhs[h],
                                start=(g == 0),
                                stop=(g == NKT - 1),
                            )
                        # g >= 3 so an oproj LDW never heads the PE queue
                        # waiting on the previous block's fresh normalization
                        if p == NPAIR - 1 and g >= 3:
                            drip_o()
                            drip_o()
                    # boundary: evacuate unnormalized attn, then normalize:
                    # 1-lane approx reciprocal on the sums row (the matmul
                    # ones-column), partition-broadcast, multiply in place
                    for h in range(2):
                        nc.vector.tensor_copy(
                            attn_sb[64 * h:64 * h + 64, p,
                                    qb * DG:(qb + 1) * DG],
                            av[h][0:64, :],
                        )
                        s0 = psmall.tile([1, DG], F32, tag="s0",
                                         name=f"s0_{qb}_{p}_{h}")
                        nc.vector.tensor_scalar_mul(
                            s0[0:1, :], av[h][64:65, :], 1.0)
                        r1 = psmall.tile([1, DG], F32, tag="r1",
                                         name=f"r1_{qb}_{p}_{h}")
                        with nc.allow_low_precision(
                                reason="softmax denominators, ~51 ULP"):
                            nc.vector.reciprocal_approx_fast(
                                r1[0:1, :], s0[0:1, :])
                        rbc = psmall.tile([P, DG], F32, tag="rbc",
                                          name=f"rbc{qb}_{p}_{h}")
                        nc.gpsimd.partition_broadcast(rbc[:], r1[0:1, :])
                        sl = attn_sb[64 * h:64 * h + 64, p,
                                     qb * DG:(qb + 1) * DG]
                        nc.vector.tensor_tensor(
                            sl, sl, rbc[64 * h:64 * h + 64, :],
                            mybir.AluOpType.mult)
                    if p == NPAIR - 1:
                        oproj_chunk(qb)
            while pending or half_open:
                drip_qk()
            while pending_o:
                drip_o()
    nc.compile()
    return nc


_CACHE = {}


def _get_nc(has_bias):
    if has_bias not in _CACHE:
        _CACHE[has_bias] = _build(has_bias)
    return _CACHE[has_bias]


def _tr(a):
    return np.ascontiguousarray(
        np.asarray(a, dtype=np.float32).T).astype(ml_dtypes.bfloat16)


def _tr8(a, mult=1.0):
    t = np.ascontiguousarray(np.asarray(a, dtype=np.float32).T) * mult
    return np.clip(t, -440.0, 440.0).astype(ml_dtypes.float8_e4m3)


def _run(Q, K, V, Wq, bq, Wk, bk, Wv, bv, Wo, bo, trace=False):
    Q, K, V = (np.asarray(t, np.float32) for t in (Q, K, V))
    Wq, Wk, Wv, Wo = (np.asarray(t, np.float32) for t in (Wq, Wk, Wv, Wo))
    bq, bk, bv, bo = (np.asarray(t, np.float32) for t in (bq, bk, bv, bo))
    B = Q.shape[0]
    has_bias = bool(np.any(bq) or np.any(bk) or np.any(bv))
    nc = _get_nc(has_bias)

    xts = [(_tr(Q[b]), _tr(K[b]), _tr(V[b])) for b in range(B)]
    wts = []
    for g in range(2):
        sl = slice(DG * g, DG * (g + 1))
        wts.append({
            "wqT": _tr(Wq[sl]), "wkT": _tr(Wk[sl]), "wvT": _tr(Wv[sl]),
            "woT": _tr(Wo[:, sl]),
            "bq": np.ascontiguousarray(bq[None, sl]).astype(ml_dtypes.bfloat16),
            "bk": np.ascontiguousarray(bk[None, sl]).astype(ml_dtypes.bfloat16),
            "bv": np.ascontiguousarray(bv[None, sl]).astype(ml_dtypes.bfloat16),
        })
    in_maps = []
    for c in range(8):
        b, g = c // 2, c % 2
        m = {
            "xqT": xts[b][0], "xkT": xts[b][1], "xvT": xts[b][2],
            "wqT": wts[g]["wqT"], "wkT": wts[g]["wkT"],
            "wvT": wts[g]["wvT"], "woT": wts[g]["woT"],
        }
        if has_bias:
            m["bq"] = wts[g]["bq"]
            m["bk"] = wts[g]["bk"]
            m["bv"] = wts[g]["bv"]
        in_maps.append(m)

    res = run_bass_kernel_spmd(nc, in_maps, core_ids=list(range(8)),
                               trace=trace)
    outp = np.empty((B, S, DM), np.float32)
    for b in range(B):
        outp[b] = (res.results[2 * b]["out"].astype(np.float32)
                   + res.results[2 * b + 1]["out"].astype(np.float32))
    outp += bo[None, None, :]
    return outp, res


def kernel(Q, K, V, Wq, bq, Wk, bk, Wv, bv, Wo, bo):
    outp, _ = _run(Q, K, V, Wq, bq, Wk, bk, Wv, bv, Wo, bo, trace=False)
    return outp

